# revision 5
# baseline (speedup 1.0000x reference)
"""Trainium2 Bass kernel for nn_BayesianOddLayer (GNN message passing).

Computation (per reference):
    total_mask = w_odd2even_mask * odd_weights              # [E, E]
    z          = (u < sigmoid(dropout_logits))              # [E]
    msg        = x @ (total_mask * z[:, None])              # [B, E]
    skip       = llr @ (w_skipconn2even_mask * llr_weights) # [B, E]
    out        = tanh(0.5 * clip(msg + skip, -10, 10))

Structure exploited: w_odd2even_mask[e1, e2] is nonzero only when
var(e1) == var(e2) (Tanner graph).  Permuting edges so each variable's
edges are contiguous makes the masked weight matrix block-diagonal with
blocks <= 128, so the [B,E]x[E,E] dense matmul collapses to ~17 small
matmuls per batch chunk.  The skip term only needs the llr rows of the
variables present in each block.  The host does pure data movement
(transpose / gather / shard); every FLOP of the reference computation
(mask multiply, sigmoid, dropout compare, matmuls, clip, tanh) runs on
device.

Sharding: data-parallel over the batch dim across 8 NeuronCores;
block weights replicated.
"""

from contextlib import ExitStack

import numpy as np

import concourse.bass as bass
import concourse.mybir as mybir
from concourse import bacc
from concourse.bass_utils import run_bass_kernel_spmd
from concourse.tile import TileContext

F32 = mybir.dt.float32
AF = mybir.ActivationFunctionType
ALU = mybir.AluOpType

B = 16384  # batch
E = 2048  # edges
NV = 512  # variable nodes
NCORES = 8
BSH = B // NCORES  # batch rows per core
CHUNK = 512  # batch columns per matmul (max fp32 moving operand)
NCHUNK = BSH // CHUNK
P = 128  # partitions


def _plan_bins(w_skipconn2even_mask: np.ndarray):
    """Group edges by variable; pack whole variables into bins of <= 128 edges.

    Returns (order, bins) where order is the edge permutation (edges sorted by
    variable) and bins is a list of (e_start, cg, v_lo, v_cnt): bin g covers
    permuted-edge slots [e_start, e_start+cg) whose variables all lie in the
    contiguous id range [v_lo, v_lo+v_cnt).
    """
    var = w_skipconn2even_mask.argmax(axis=0).astype(np.int64)  # [E]
    order = np.argsort(var, kind="stable")
    uniq, counts = np.unique(var[order], return_counts=True)
    assert counts.max() <= P, "single variable with degree > 128"
    bins = []
    e_pos = 0
    i = 0
    while i < len(uniq):
        cg = 0
        v_lo = int(uniq[i])
        v_hi = v_lo
        start = e_pos
        while i < len(uniq) and cg + counts[i] <= P:
            cg += int(counts[i])
            v_hi = int(uniq[i])
            i += 1
        bins.append((start, cg, v_lo, v_hi - v_lo + 1))
        e_pos += cg
    assert e_pos == E
    assert all(vc <= P for (_, _, _, vc) in bins)
    return order, bins


def _build_nc(bins):
    NB = len(bins)
    nc = bacc.Bacc("TRN2", target_bir_lowering=False, debug=False,
                   num_devices=NCORES)
    xt = nc.dram_tensor("xt", [E, BSH], F32, kind="ExternalInput").ap()
    llrt = nc.dram_tensor("llrt", [NV, BSH], F32, kind="ExternalInput").ap()
    wblk = nc.dram_tensor("wblk", [P, NB * P], F32, kind="ExternalInput").ap()
    mblk = nc.dram_tensor("mblk", [P, NB * P], F32, kind="ExternalInput").ap()
    swblk = nc.dram_tensor("swblk", [P, NB * P], F32, kind="ExternalInput").ap()
    smblk = nc.dram_tensor("smblk", [P, NB * P], F32, kind="ExternalInput").ap()
    ublk = nc.dram_tensor("ublk", [P, NB], F32, kind="ExternalInput").ap()
    lgblk = nc.dram_tensor("lgblk", [P, NB], F32, kind="ExternalInput").ap()
    outt = nc.dram_tensor("outt", [E, BSH], F32, kind="ExternalOutput").ap()

    with TileContext(nc) as tc, ExitStack() as ctx:
        cpool = ctx.enter_context(tc.tile_pool(name="const", bufs=1))

        # z = (u < sigmoid(dropout_logits)), laid out per (bin row, bin)
        ub = cpool.tile([P, NB], F32)
        nc.sync.dma_start(ub[:], ublk)
        zt = cpool.tile([P, NB], F32)
        nc.sync.dma_start(zt[:], lgblk)
        nc.scalar.activation(zt[:], zt[:], AF.Sigmoid)
        nc.vector.tensor_tensor(zt[:], ub[:], zt[:], ALU.is_lt)

        # Weff blocks = odd_weights * mask * z[row]
        wt = cpool.tile([P, NB * P], F32)
        nc.sync.dma_start(wt[:], wblk)
        mt = cpool.tile([P, NB * P], F32)
        nc.sync.dma_start(mt[:], mblk)
        nc.vector.tensor_mul(wt[:], wt[:], mt[:])
        for g in range(NB):
            sl = wt[:, g * P : (g + 1) * P]
            nc.vector.tensor_scalar(sl, sl, zt[:, g : g + 1], None, ALU.mult)

        # skip blocks = llr_weights * skip_mask
        st = cpool.tile([P, NB * P], F32)
        nc.sync.dma_start(st[:], swblk)
        nc.sync.dma_start(mt[:], smblk)  # reuse mt slot
        nc.vector.tensor_mul(st[:], st[:], mt[:])

        xpool = ctx.enter_context(tc.tile_pool(name="xp", bufs=6))
        lpool = ctx.enter_context(tc.tile_pool(name="lp", bufs=6))
        pspool = ctx.enter_context(tc.tile_pool(name="ps", bufs=6, space="PSUM"))
        opool = ctx.enter_context(tc.tile_pool(name="op", bufs=6))

        for g, (es, cg, vlo, vcnt) in enumerate(bins):
            for nb in range(NCHUNK):
                c0 = nb * CHUNK
                xtile = xpool.tile([cg, CHUNK], F32)
                nc.sync.dma_start(xtile[:], xt[es : es + cg, c0 : c0 + CHUNK])
                ltile = lpool.tile([vcnt, CHUNK], F32)
                nc.sync.dma_start(ltile[:], llrt[vlo : vlo + vcnt, c0 : c0 + CHUNK])
                ps = pspool.tile([cg, CHUNK], F32)
                nc.tensor.matmul(
                    ps[:], wt[0:cg, g * P : g * P + cg], xtile[:],
                    start=True, stop=False,
                )
                nc.tensor.matmul(
                    ps[:], st[0:vcnt, g * P : g * P + cg], ltile[:],
                    start=False, stop=True,
                )
                ot = opool.tile([cg, CHUNK], F32)
                nc.vector.tensor_scalar(ot[:], ps[:], 10.0, -10.0, ALU.min, ALU.max)
                nc.scalar.activation(ot[:], ot[:], AF.Tanh, scale=0.5)
                nc.sync.dma_start(outt[es : es + cg, c0 : c0 + CHUNK], ot[:])
    nc.compile()
    return nc


def _prep(x, llr, u, odd_weights, llr_weights, dropout_logits,
          w_odd2even_mask, w_skipconn2even_mask):
    """Host-side data movement: edge permutation, block gathers, shards."""
    order, bins = _plan_bins(np.asarray(w_skipconn2even_mask))
    NB = len(bins)

    ow = np.asarray(odd_weights, np.float32)
    msk = np.asarray(w_odd2even_mask, np.float32)
    lw = np.asarray(llr_weights, np.float32)
    smask = np.asarray(w_skipconn2even_mask, np.float32)
    u = np.asarray(u, np.float32)
    lg = np.asarray(dropout_logits, np.float32)

    wblk = np.zeros((P, NB * P), np.float32)
    mblk = np.zeros((P, NB * P), np.float32)
    swblk = np.zeros((P, NB * P), np.float32)
    smblk = np.zeros((P, NB * P), np.float32)
    ublk = np.full((P, NB), 2.0, np.float32)  # pad > 1 so z=0 on pad rows
    lgblk = np.zeros((P, NB), np.float32)
    for g, (es, cg, vlo, vcnt) in enumerate(bins):
        pe = order[es : es + cg]
        wblk[:cg, g * P : g * P + cg] = ow[np.ix_(pe, pe)]
        mblk[:cg, g * P : g * P + cg] = msk[np.ix_(pe, pe)]
        swblk[:vcnt, g * P : g * P + cg] = lw[vlo : vlo + vcnt][:, pe]
        smblk[:vcnt, g * P : g * P + cg] = smask[vlo : vlo + vcnt][:, pe]
        ublk[:cg, g] = u[pe]
        lgblk[:cg, g] = lg[pe]

    x = np.asarray(x, np.float32)
    llr = np.asarray(llr, np.float32)
    in_maps = []
    for c in range(NCORES):
        sl = slice(c * BSH, (c + 1) * BSH)
        in_maps.append({
            "xt": np.ascontiguousarray(x[sl].T[order]),
            "llrt": np.ascontiguousarray(llr[sl].T),
            "wblk": wblk, "mblk": mblk, "swblk": swblk, "smblk": smblk,
            "ublk": ublk, "lgblk": lgblk,
        })
    return order, bins, in_maps


def _run(inputs: dict, trace: bool = False, **kwargs):
    order, bins, in_maps = _prep(**inputs)
    nc = _build_nc(bins)
    res = run_bass_kernel_spmd(nc, in_maps, list(range(NCORES)), trace=trace, **kwargs)
    out = np.empty((B, E), np.float32)
    for c in range(NCORES):
        sl = slice(c * BSH, (c + 1) * BSH)
        out[sl][:, order] = res.results[c]["outt"].T
    return out, res


def kernel(**inputs) -> np.ndarray:
    out, _ = _run(inputs, trace=False)
    return out


# revision 6
# speedup vs baseline: 3.7036x; 3.7036x over previous
"""Trainium2 Bass kernel for nn_BayesianOddLayer (GNN message passing).

Computation (per reference):
    total_mask = w_odd2even_mask * odd_weights              # [E, E]
    z          = (u < sigmoid(dropout_logits))              # [E]
    msg        = x @ (total_mask * z[:, None])              # [B, E]
    skip       = llr @ (w_skipconn2even_mask * llr_weights) # [B, E]
    out        = tanh(0.5 * clip(msg + skip, -10, 10))

Structure exploited: w_odd2even_mask[e1, e2] is nonzero only when
var(e1) == var(e2) (Tanner graph), and the skip term feeds each edge
from exactly its own variable.  Packing each bin with a set of whole
variables — all their edges plus one llr row per variable, <= 128 rows
total — turns the two matmuls into ONE small matmul per bin:
    lhsT rows  = [edges of bin (Weff block) ; variables of bin (skip block)]
    rhs rows   = [x^T rows of those edges   ; llr^T rows of those vars  ]
so the dense [B,E]x[E,E] + [B,N]x[N,E] work collapses to ~20 K=128
matmuls per batch chunk.  The host does pure data movement (gather /
transpose / pad / shard); every FLOP of the reference computation (mask
multiply, sigmoid, dropout compare, matmul, clip, tanh) runs on device.

DMA layout: rhs and out live in DRAM as [128, NCHUNK, NB, CHUNK] so each
batch chunk is ONE multi-MB DMA with large per-partition contiguous
runs (the single-DMA 16-engine split gives ~340+ GB/s only for large
transfers).

Sharding: data-parallel over the batch dim across 8 NeuronCores;
block weights replicated.
"""

from contextlib import ExitStack

import numpy as np

import concourse.bass as bass
import concourse.mybir as mybir
from concourse import bacc
from concourse.bass_utils import run_bass_kernel_spmd
from concourse.tile import TileContext

F32 = mybir.dt.float32
AF = mybir.ActivationFunctionType
ALU = mybir.AluOpType

B = 16384  # batch
E = 2048  # edges
NV = 512  # variable nodes
NCORES = 8
BSH = B // NCORES  # batch rows per core
CHUNK = 512  # batch columns per matmul (max fp32 moving operand)
NCHUNK = BSH // CHUNK
P = 128  # partitions


def _plan_bins(w_skipconn2even_mask: np.ndarray):
    """Pack whole variables into bins: per variable, deg(v) edge rows plus
    one llr row, while total rows <= 128.

    Returns a list of (edge_ids, var_ids) per bin.
    """
    var = w_skipconn2even_mask.argmax(axis=0).astype(np.int64)  # [E]
    edges_of = [np.where(var == v)[0] for v in range(NV)]
    bins = []
    cur_e, cur_v, used = [], [], 0
    for v in range(NV):
        need = len(edges_of[v]) + 1
        if len(edges_of[v]) == 0:
            continue
        assert need <= P
        if used + need > P:
            bins.append((np.concatenate(cur_e), np.array(cur_v)))
            cur_e, cur_v, used = [], [], 0
        cur_e.append(edges_of[v])
        cur_v.append(v)
        used += need
    if cur_v:
        bins.append((np.concatenate(cur_e), np.array(cur_v)))
    assert sum(len(e) for e, _ in bins) == E
    return bins


def _build_nc(NB):
    nc = bacc.Bacc("TRN2", target_bir_lowering=False, debug=False,
                   num_devices=NCORES)
    W = NB * CHUNK  # free-dim width of one chunk's rhs/out tile
    rhsp = nc.dram_tensor("rhsp", [P, NCHUNK * W], F32, kind="ExternalInput").ap()
    wcomb = nc.dram_tensor("wcomb", [P, NB * P], F32, kind="ExternalInput").ap()
    mcomb = nc.dram_tensor("mcomb", [P, NB * P], F32, kind="ExternalInput").ap()
    ucomb = nc.dram_tensor("ucomb", [P, NB], F32, kind="ExternalInput").ap()
    lgcomb = nc.dram_tensor("lgcomb", [P, NB], F32, kind="ExternalInput").ap()
    outp = nc.dram_tensor("outp", [P, NCHUNK * W], F32, kind="ExternalOutput").ap()

    with TileContext(nc) as tc, ExitStack() as ctx:
        cpool = ctx.enter_context(tc.tile_pool(name="const", bufs=1))

        # z = (u < sigmoid(dropout_logits)); var rows have u=-1 -> z=1
        ub = cpool.tile([P, NB], F32)
        nc.sync.dma_start(ub[:], ucomb)
        zt = cpool.tile([P, NB], F32)
        nc.sync.dma_start(zt[:], lgcomb)
        nc.scalar.activation(zt[:], zt[:], AF.Sigmoid)
        nc.vector.tensor_tensor(zt[:], ub[:], zt[:], ALU.is_lt)

        # combined blocks: [edge rows: odd_weights*mask*z ; var rows: llr_w*smask]
        wt = cpool.tile([P, NB * P], F32)
        nc.sync.dma_start(wt[:], wcomb)
        mt = cpool.tile([P, NB * P], F32)
        nc.sync.dma_start(mt[:], mcomb)
        nc.vector.tensor_mul(wt[:], wt[:], mt[:])
        for g in range(NB):
            sl = wt[:, g * P : (g + 1) * P]
            nc.vector.tensor_scalar(sl, sl, zt[:, g : g + 1], None, ALU.mult)

        rpool = ctx.enter_context(tc.tile_pool(name="rhs", bufs=2))
        opool = ctx.enter_context(tc.tile_pool(name="out", bufs=2))
        pspool = ctx.enter_context(tc.tile_pool(name="ps", bufs=4, space="PSUM"))

        for nb in range(NCHUNK):
            rt = rpool.tile([P, W], F32)
            nc.sync.dma_start(rt[:], rhsp[:, nb * W : (nb + 1) * W])
            ot = opool.tile([P, W], F32)
            for g in range(NB):
                ps = pspool.tile([P, CHUNK], F32)
                nc.tensor.matmul(
                    ps[:], wt[:, g * P : (g + 1) * P],
                    rt[:, g * CHUNK : (g + 1) * CHUNK],
                    start=True, stop=True,
                )
                sl = ot[:, g * CHUNK : (g + 1) * CHUNK]
                nc.vector.tensor_scalar(sl, ps[:], 10.0, -10.0, ALU.min, ALU.max)
                nc.scalar.activation(sl, sl, AF.Tanh, scale=0.5)
            nc.scalar.dma_start(outp[:, nb * W : (nb + 1) * W], ot[:])
    nc.compile()
    return nc


def _prep(x, llr, u, odd_weights, llr_weights, dropout_logits,
          w_odd2even_mask, w_skipconn2even_mask):
    """Host-side data movement: bin packing, block gathers, shards."""
    ow = np.asarray(odd_weights, np.float32)
    msk = np.asarray(w_odd2even_mask, np.float32)
    lw = np.asarray(llr_weights, np.float32)
    smask = np.asarray(w_skipconn2even_mask, np.float32)
    u = np.asarray(u, np.float32)
    lg = np.asarray(dropout_logits, np.float32)

    bins = _plan_bins(smask)
    NB = len(bins)

    wcomb = np.zeros((P, NB * P), np.float32)
    mcomb = np.zeros((P, NB * P), np.float32)
    ucomb = np.full((P, NB), 2.0, np.float32)  # pad rows: z=0 (unused anyway)
    lgcomb = np.zeros((P, NB), np.float32)
    # rhs row r = g*128+p sources from concat(x^T, llr^T, zero-row)
    rows_src = np.full(NB * P, E + NV, np.int64)
    for g, (pe, vs) in enumerate(bins):
        cg, nv = len(pe), len(vs)
        c = g * P
        wcomb[:cg, c : c + cg] = ow[np.ix_(pe, pe)]
        wcomb[cg : cg + nv, c : c + cg] = lw[np.ix_(vs, pe)]
        mcomb[:cg, c : c + cg] = msk[np.ix_(pe, pe)]
        mcomb[cg : cg + nv, c : c + cg] = smask[np.ix_(vs, pe)]
        ucomb[:cg, g] = u[pe]
        ucomb[cg : cg + nv, g] = -1.0  # var rows: z=1 (no dropout on skip)
        lgcomb[:cg, g] = lg[pe]
        rows_src[c : c + cg] = pe
        rows_src[c + cg : c + cg + nv] = E + vs

    x = np.asarray(x, np.float32)
    llr = np.asarray(llr, np.float32)
    in_maps = []
    for c in range(NCORES):
        sl = slice(c * BSH, (c + 1) * BSH)
        base = np.concatenate(
            [x[sl].T, llr[sl].T, np.zeros((1, BSH), np.float32)], axis=0
        )
        rhs = base[rows_src]  # [NB*128, BSH]
        rhsp = np.ascontiguousarray(
            rhs.reshape(NB, P, NCHUNK, CHUNK).transpose(1, 2, 0, 3)
        ).reshape(P, NCHUNK * NB * CHUNK)
        in_maps.append({
            "rhsp": rhsp, "wcomb": wcomb, "mcomb": mcomb,
            "ucomb": ucomb, "lgcomb": lgcomb,
        })
    return bins, in_maps


def _run(inputs: dict, trace: bool = False, **kwargs):
    bins, in_maps = _prep(**inputs)
    NB = len(bins)
    nc = _build_nc(NB)
    res = run_bass_kernel_spmd(nc, in_maps, list(range(NCORES)), trace=trace, **kwargs)

    # decode: outp [128, NCHUNK, NB, CHUNK] -> rows (g, p) -> edge column
    valid = np.zeros(NB * P, bool)
    dest = np.zeros(NB * P, np.int64)
    for g, (pe, _) in enumerate(bins):
        valid[g * P : g * P + len(pe)] = True
        dest[g * P : g * P + len(pe)] = pe
    out = np.empty((B, E), np.float32)
    for c in range(NCORES):
        sl = slice(c * BSH, (c + 1) * BSH)
        arr = (res.results[c]["outp"]
               .reshape(P, NCHUNK, NB, CHUNK)
               .transpose(2, 0, 1, 3)
               .reshape(NB * P, BSH))
        out[sl][:, dest[valid]] = arr[valid].T
    return out, res


def kernel(**inputs) -> np.ndarray:
    out, _ = _run(inputs, trace=False)
    return out
